# revision 19
# baseline (speedup 1.0000x reference)
"""BertAttention (QKV + MHA + output proj + residual + LayerNorm) on 8 TRN2 cores.

Sharding: heads 2c,2c+1 -> core c (tensor-parallel attention); output
projection + LayerNorm token-sharded after an on-device AllToAll of the
normalized per-head context.  Token ownership is INTERLEAVED across batches
(core c owns b0 tokens [256c,256c+256) and b1 tokens [2048+256c,+256)) so the
AllToAll splits into two halves, the first overlapping batch-1 attention.

Attention layout: activations transposed ([feature, token]).  Scores run in
S^T ([key, query]) with the head pair packed into the PE array.  Softmax exp
is split across THREE engines: ACT (true exp) and DVE/Pool (Schraudolph
bit-trick exp: int16(x*128*log2e + C) bitcast to bf16).  The context matmul
is FLIPPED: stationary = exp-scores chunk [128k x 128q], moving = V^T
augmented with a ones column [128 x 65] -> PSUM [128q, 64chan + z], costing
65 PE rows instead of 512 and delivering softmax denominators z in the same
partition as their queries, so normalization is a per-partition
tensor_scalar multiply.  The a2a payload is [token, chan]; the receive side
uses DMA crossbar transposes to restore [chan, token] for the output dense.
"""
import sys

sys.path.insert(0, "/opt/trn_rl_repo")

import numpy as np
import ml_dtypes

import concourse.bacc as bacc
import concourse.mybir as mybir
import concourse.tile as tile
from concourse.bass_utils import run_bass_kernel_spmd
from concourse.masks import make_identity

B, S, H = 2, 2048, 1024
NH, HD = 16, 64
W = 8                    # cores
T = B * S                # 4096 flat tokens
TOK = T // W             # 512 tokens owned per core (256 per batch)
HTOK = TOK // 2          # 256: tokens owned per batch-half
CPC = (NH // W) * HD     # 128 ctx channels per core (2 heads)
QT = 512                 # query tile
NQT = S // QT            # 4 query tiles per batch
NKC = S // 128           # 16 key chunks per batch
NK = H // 128            # 8 contraction chunks for the projections
NSL = T // HTOK          # 16 a2a slots of 256 tokens

F32 = mybir.dt.float32
BF16 = mybir.dt.bfloat16
I16 = mybir.dt.int16
FP8 = mybir.dt.float8e4
BF = ml_dtypes.bfloat16
F8 = ml_dtypes.float8_e4m3

LOG2E = 1.4426950408889634
SCHRAUD_C = 127.0 * 128.0 - 6.87   # magic + error centering
WS = 32.0                          # host prescale on Wq/Wk/Wv for fp8

_NC_CACHE = {}


def build_nc(reps=1, fp8=True, exp_split=(10, 6, 0), no_collective=False, sc8=True):
    """exp_split = (#ACT, #DVE, #Pool) j-chunks out of NKC=16 per query tile."""
    assert sum(exp_split) == NKC
    nc = bacc.Bacc(None)

    if fp8:
        xdr = nc.dram_tensor("xdr", [64, NK, 2, T], FP8, kind="ExternalInput")
        wq = nc.dram_tensor("wq", [64, NK, 2, CPC], FP8, kind="ExternalInput")
        wk = nc.dram_tensor("wk", [64, NK, 2, CPC], FP8, kind="ExternalInput")
        wv = nc.dram_tensor("wv", [64, NK, 2, CPC], FP8, kind="ExternalInput")
    else:
        xdr = nc.dram_tensor("xdr", [H, T], BF16, kind="ExternalInput")
        wq = nc.dram_tensor("wq", [H, CPC], BF16, kind="ExternalInput")
        wk = nc.dram_tensor("wk", [H, CPC], BF16, kind="ExternalInput")
        wv = nc.dram_tensor("wv", [H, CPC], BF16, kind="ExternalInput")
    bq = nc.dram_tensor("bq", [CPC], F32, kind="ExternalInput")
    bk = nc.dram_tensor("bk", [CPC], F32, kind="ExternalInput")
    bv = nc.dram_tensor("bv", [CPC], F32, kind="ExternalInput")
    wo = nc.dram_tensor("wo", [H, H], BF16, kind="ExternalInput")
    bo = nc.dram_tensor("bo", [H], F32, kind="ExternalInput")
    gamma = nc.dram_tensor("gamma", [H], F32, kind="ExternalInput")
    beta = nc.dram_tensor("beta", [H], F32, kind="ExternalInput")
    hT = nc.dram_tensor("hT", [H, TOK], F32, kind="ExternalInput")
    maskT = nc.dram_tensor("maskT", [B, S], F32, kind="ExternalInput")
    maskD = nc.dram_tensor("maskD", [B, S], F32, kind="ExternalInput")
    y = nc.dram_tensor("y", [H, TOK], F32, kind="ExternalOutput")

    # exp-argument scaling: scores carry WS^2 from the weight prescale (fp8)
    sact = 0.125 / (WS * WS) if fp8 else 0.125
    alpha_dr = 128.0 * LOG2E * sact
    ones_val = WS if fp8 else 1.0   # va ones-column; makes z carry the same
    #                                 scale as the WS-scaled context values

    from contextlib import ExitStack
    with tile.TileContext(nc) as tc, ExitStack() as _stk:
        constp = _stk.enter_context(tc.tile_pool(name="const", bufs=1))
        wpool = _stk.enter_context(tc.tile_pool(name="weights", bufs=1))
        xtp = _stk.enter_context(tc.tile_pool(name="xt", bufs=1))
        qkp = _stk.enter_context(tc.tile_pool(name="qk", bufs=1))
        qrp = _stk.enter_context(tc.tile_pool(name="qr", bufs=2))
        vsp = _stk.enter_context(tc.tile_pool(name="vstage", bufs=2))
        vap = _stk.enter_context(tc.tile_pool(name="vaug", bufs=32))
        expp = _stk.enter_context(tc.tile_pool(name="exps", bufs=6))
        zp = _stk.enter_context(tc.tile_pool(name="znorm", bufs=8))
        stp = _stk.enter_context(tc.tile_pool(name="stage", bufs=4))
        dramp = _stk.enter_context(tc.tile_pool(name="dram", bufs=1, space="DRAM"))
        lnp = _stk.enter_context(tc.tile_pool(name="ln", bufs=2))
        xtl = _stk.enter_context(tc.tile_pool(name="xtile", bufs=NK))
        cop = _stk.enter_context(tc.tile_pool(name="ctxown", bufs=2))

        ident = constp.tile([128, 128], BF16)
        make_identity(nc, ident[:])
        ones128 = constp.tile([128, 1], BF16)
        nc.vector.memset(ones128[:], 1.0)
        ones128f = constp.tile([128, 1], F32)
        nc.vector.memset(ones128f[:], 1.0)
        ones128r = constp.tile([128, 1], mybir.dt.float32r)
        nc.vector.tensor_copy(ones128r[:], ones128f[:])
        a2a_in = [
            dramp.tile([W, HTOK, CPC], BF16, tag=f"a2a_in{i}", name=f"a2a_in{i}")
            for i in range(B)
        ]
        a2a_out = [
            dramp.tile([W, HTOK, CPC], BF16, tag=f"a2a_out{i}", name=f"a2a_out{i}")
            for i in range(B)
        ]

        for rep in range(reps):
            biases = constp.tile([128, 3], F32, tag="biases", bufs=min(reps, 2))
            nc.sync.dma_start(out=biases[:, 0:1], in_=bq[:].unsqueeze(1))
            nc.sync.dma_start(out=biases[:, 1:2], in_=bk[:].unsqueeze(1))
            nc.sync.dma_start(out=biases[:, 2:3], in_=bv[:].unsqueeze(1))
            mask_sb = constp.tile([128, B, NKC], F32, tag="mask", bufs=min(reps, 2))
            nc.sync.dma_start(
                out=mask_sb[:, :, :], in_=maskT.rearrange("b (j p) -> p b j", p=128)
            )
            maskd_sb = constp.tile([128, B, NKC], F32, tag="maskd", bufs=min(reps, 2))
            nc.sync.dma_start(
                out=maskd_sb[:, :, :], in_=maskD.rearrange("b (j p) -> p b j", p=128)
            )
            if fp8:
                wq_sb = wpool.tile([64, NK, 2, CPC], FP8, tag="wq", bufs=1)
                wk_sb = wpool.tile([64, NK, 2, CPC], FP8, tag="wk", bufs=1)
                wv_sb = wpool.tile([64, NK, 2, CPC], FP8, tag="wv", bufs=1)
                for w_dram, w_sb in ((wq, wq_sb), (wk, wk_sb), (wv, wv_sb)):
                    nc.sync.dma_start(out=w_sb[:, :, :, :], in_=w_dram[:, :, :, :])
            else:
                wq_sb = wpool.tile([128, NK, CPC], BF16, tag="wq", bufs=1)
                wk_sb = wpool.tile([128, NK, CPC], BF16, tag="wk", bufs=1)
                wv_sb = wpool.tile([128, NK, CPC], BF16, tag="wv", bufs=1)
                for w_dram, w_sb in ((wq, wq_sb), (wk, wk_sb), (wv, wv_sb)):
                    wre = w_dram.rearrange("(c p) m -> p c m", p=128)
                    nc.sync.dma_start(out=w_sb[:, :, :], in_=wre[:, :, :])
            def load_tail_data():
                # tail-only data: emitted after batch-0 QKV so these DMAs
                # ride behind the attention compute instead of delaying it
                wo_sb = wpool.tile([128, NK, H], BF16, tag="wo", bufs=1)
                nc.sync.dma_start(
                    out=wo_sb[:, :, :], in_=wo.rearrange("(c p) m -> p c m", p=128)
                )
                obg = constp.tile([128, NK, 3], F32, tag="obg", bufs=min(reps, 2))
                nc.sync.dma_start(
                    out=obg[:, :, 0], in_=bo.rearrange("(c p) -> p c", p=128)
                )
                nc.sync.dma_start(
                    out=obg[:, :, 1], in_=gamma.rearrange("(c p) -> p c", p=128)
                )
                nc.sync.dma_start(
                    out=obg[:, :, 2], in_=beta.rearrange("(c p) -> p c", p=128)
                )
                return wo_sb, obg

            with ExitStack() as _ps_stk:
                # PSUM: proj 2 banks + sp 4 banks + cx 2 banks = 8
                proj_ps = _ps_stk.enter_context(
                    tc.tile_pool(name=f"proj_ps{rep}", bufs=2, space="PSUM")
                )
                sp_ps = _ps_stk.enter_context(
                    tc.tile_pool(name=f"sp_ps{rep}", bufs=2, space="PSUM")
                )
                cx_ps = _ps_stk.enter_context(
                    tc.tile_pool(name=f"cx_ps{rep}", bufs=1, space="PSUM")
                )
                def load_x(b):
                    if fp8:
                        x8 = xtp.tile([64, NK, 2, S], FP8, tag="x8")
                        nc.sync.dma_start(
                            out=x8[:, :, :, 0:S // 2],
                            in_=xdr[:, :, :, b * S:b * S + S // 2],
                        )
                        nc.sync.dma_start(
                            out=x8[:, :, :, S // 2:S],
                            in_=xdr[:, :, :, b * S + S // 2:(b + 1) * S],
                        )
                        return x8
                    xt_tiles = []
                    for k in range(NK):
                        xt_t = xtp.tile([128, S], BF16, tag="xt", bufs=2 * NK)
                        nc.sync.dma_start(
                            out=xt_t[:, 0:S // 2],
                            in_=xdr[k * 128:(k + 1) * 128, b * S:b * S + S // 2],
                        )
                        nc.sync.dma_start(
                            out=xt_t[:, S // 2:S],
                            in_=xdr[k * 128:(k + 1) * 128,
                                    b * S + S // 2:(b + 1) * S],
                        )
                        xt_tiles.append(xt_t)
                    return xt_tiles

                def do_qkv(b, xin):
                    if fp8:
                        x8 = xin
                    else:
                        xt_tiles = xin
                    qdt = FP8 if sc8 else BF16
                    qTt = qkp.tile([128, S], qdt, tag="qT")
                    kTt = qkp.tile([128, S], qdt, tag="kT")
                    vaug_tiles = []
                    for t in range(NQT):
                        tsl = slice(t * QT, (t + 1) * QT)
                        for w_sb, bcol, dstT in (
                            (wq_sb, 0, qTt), (wk_sb, 1, kTt), (wv_sb, 2, None)
                        ):
                            ps = proj_ps.tile([128, QT], F32, tag="proj")
                            if fp8:
                                for kk in range(NK):
                                    nc.tensor.matmul(
                                        ps[:, :],
                                        w_sb[:, kk, :, :],
                                        x8[:, kk, :, tsl],
                                        start=(kk == 0),
                                        stop=(kk == NK - 1),
                                        perf_mode=mybir.MatmulPerfMode.DoubleRow,
                                    )
                            else:
                                for k in range(NK):
                                    nc.tensor.matmul(
                                        ps[:, :],
                                        w_sb[:, k, :],
                                        xt_tiles[k][:, tsl],
                                        start=(k == 0),
                                        stop=(k == NK - 1),
                                    )
                            if dstT is not None:
                                nc.vector.tensor_scalar_add(
                                    dstT[:, tsl], ps[:, :], biases[:, bcol:bcol + 1]
                                )
                            else:
                                vst = vsp.tile([128, QT], BF16, tag="vst")
                                nc.vector.tensor_scalar_add(
                                    vst[:, :], ps[:, :], biases[:, bcol:bcol + 1]
                                )
                                for s4 in range(QT // 128):
                                    vps = proj_ps.tile([128, 128], BF16, tag="proj")
                                    nc.tensor.transpose(
                                        vps[:, :],
                                        vst[:, s4 * 128:(s4 + 1) * 128],
                                        ident[:, :],
                                    )
                                    va = vap.tile([128, 130], BF16, tag="vaug")
                                    nc.vector.memset(va[:, 64:65], ones_val)
                                    nc.vector.memset(va[:, 129:130], ones_val)
                                    nc.vector.tensor_copy(va[:, 0:64], vps[:, 0:64])
                                    nc.vector.tensor_copy(va[:, 65:129], vps[:, 64:128])
                                    vaug_tiles.append(va)
                    if not sc8:
                        return qTt, kTt, vaug_tiles
                    qr = qrp.tile([32, 2, 2, S], FP8, tag="qr")
                    kr = qrp.tile([32, 2, 2, S], FP8, tag="kr")
                    for h in range(2):
                        for g in range(2):
                            base = h * 64 + g * 32
                            nc.sync.dma_start(
                                out=qr[:, h, g, :], in_=qTt[base:base + 32, :]
                            )
                            nc.sync.dma_start(
                                out=kr[:, h, g, :], in_=kTt[base:base + 32, :]
                            )
                    return qr, kr, vaug_tiles

                def do_attn(b, qTt, kTt, vaug_tiles):
                    # -------- attention --------
                    # interleave exp engines so no engine's chunks cluster
                    acts, dves, pools = exp_split
                    eng_sched = []
                    ca = cd = cp = 0
                    for j in range(NKC):
                        # pick the engine furthest behind its quota
                        cand = [
                            (ca / acts if acts else 9, "act"),
                            (cd / dves if dves else 9, "dve"),
                            (cp / pools if pools else 9, "pool"),
                        ]
                        eng = min(cand)[1]
                        eng_sched.append(eng)
                        ca, cd, cp = (
                            ca + (eng == "act"),
                            cd + (eng == "dve"),
                            cp + (eng == "pool"),
                        )
                    LAG = 3
                    for t in range(NQT):
                        tsl = slice(t * QT, (t + 1) * QT)
                        cx = cx_ps.tile([128, 8, 128], F32, tag="cx")
                        pend = []

                        def emit_ctx(j, es, va):
                            # one accumulation group per PSUM bank (slots 0-3
                            # = bank0, 4-7 = bank1): start/stop zero-regions
                            # are 2KB, so per-slot groups would corrupt
                            for h in range(2):
                                for qc in range(4):
                                    nc.tensor.matmul(
                                        cx[:, h * 4 + qc, 0:65],
                                        es[:, h, qc * 128:(qc + 1) * 128],
                                        va[:, h * 65:h * 65 + 65],
                                        start=(j == 0 and qc == 0),
                                        stop=(j == NKC - 1 and qc == 3),
                                        skip_group_check=True,
                                    )

                        for j in range(NKC):
                            jsl = slice(j * 128, (j + 1) * 128)
                            sp = sp_ps.tile([128, 2, QT], F32, tag="sc")
                            if sc8:
                                for h in range(2):
                                    nc.tensor.matmul(
                                        sp[:, h, :], kTt[:, h, :, jsl],
                                        qTt[:, h, :, tsl],
                                        start=True, stop=True,
                                        perf_mode=mybir.MatmulPerfMode.DoubleRow,
                                    )
                            else:
                                nc.tensor.matmul(
                                    sp[:, 0, :], kTt[0:64, jsl], qTt[0:64, tsl],
                                    start=True, stop=True, tile_position=(0, 0),
                                )
                                nc.tensor.matmul(
                                    sp[:, 1, :], kTt[64:128, jsl], qTt[64:128, tsl],
                                    start=True, stop=True, tile_position=(64, 0),
                                )
                            es = expp.tile([128, 2, QT], BF16, tag="es")
                            eng = eng_sched[j]
                            if eng == "act":
                                nc.scalar.activation(
                                    es[:, :, :], sp[:, :, :],
                                    mybir.ActivationFunctionType.Exp,
                                    bias=mask_sb[:, b, j:j + 1], scale=sact,
                                )
                            elif eng == "dve":
                                nc.vector.tensor_scalar(
                                    es[:, :, :].bitcast(I16), sp[:, :, :],
                                    alpha_dr, maskd_sb[:, b, j:j + 1],
                                    op0=mybir.AluOpType.mult,
                                    op1=mybir.AluOpType.add,
                                )
                            else:  # Pool cannot access PSUM on HW
                                nc.vector.tensor_scalar(
                                    es[:, :, :].bitcast(I16), sp[:, :, :],
                                    alpha_dr, maskd_sb[:, b, j:j + 1],
                                    op0=mybir.AluOpType.mult,
                                    op1=mybir.AluOpType.add,
                                )
                            pend.append((j, es, vaug_tiles[j]))
                            if len(pend) > LAG:
                                emit_ctx(*pend.pop(0))
                        for item in pend:
                            emit_ctx(*item)
                        # normalize + stage to a2a
                        for qc in range(4):
                            st = stp.tile([128, 128], BF16, tag="st")
                            for h in range(2):
                                sl = h * 4 + qc
                                r = zp.tile([128, 1], F32, tag="r")
                                nc.vector.reciprocal_approx_fast(
                                    r[:, :], cx[:, sl, 64:65]
                                )
                                eng = nc.vector
                                eng.tensor_scalar_mul(
                                    st[:, h * 64:(h + 1) * 64].bitcast(BF16),
                                    cx[:, sl, 0:64], r[:, 0:1],
                                )
                            tok0 = t * QT + qc * 128
                            sl_ix, off = tok0 // HTOK, tok0 % HTOK
                            nc.sync.dma_start(
                                out=a2a_in[b][sl_ix, off:off + 128, :], in_=st[:, :]
                            )
                    if no_collective:
                        for i in range(W):
                            nc.sync.dma_start(
                                out=a2a_out[b][i, :, :], in_=a2a_in[b][i, :, :]
                            )
                    else:
                        nc.gpsimd.collective_compute(
                            "AllToAll",
                            mybir.AluOpType.bypass,
                            replica_groups=[list(range(W))],
                            ins=[a2a_in[b][:, :, :].opt()],
                            outs=[a2a_out[b][:, :, :].opt()],
                        )

                x0 = load_x(0)
                s0 = do_qkv(0, x0)
                x1 = load_x(1)                        # prefetch batch 1
                wo_sb, obg = load_tail_data()         # rides behind attention
                do_attn(0, *s0)
                s1 = do_qkv(1, x1)
                do_attn(1, *s1)

            # ---- output projection + residual + LayerNorm (both halves) ----
            with ExitStack() as _op_stk:
                op_ps = _op_stk.enter_context(
                    tc.tile_pool(name=f"op_ps{rep}", bufs=2, space="PSUM")
                )
                mom_ps = _op_stk.enter_context(
                    tc.tile_pool(name=f"mom_ps{rep}", bufs=2, space="PSUM")
                )
                for hf in range(2):
                    ctx_own = cop.tile([128, NK, HTOK], BF16, tag="ctxown")
                    for i in range(W):
                        nc.sync.dma_start(
                            out=ctx_own[:, i, :],
                            in_=a2a_out[hf][i, :, :],
                            transpose=True,
                        )
                    hs = slice(hf * HTOK, (hf + 1) * HTOK)
                    hT_sb = wpool.tile([128, NK, HTOK], F32, tag="hT", bufs=1)
                    nc.sync.dma_start(
                        out=hT_sb[:, :, :],
                        in_=hT.rearrange("(c p) m -> p c m", p=128)[:, :, hs],
                    )
                    mom1 = mom_ps.tile([1, HTOK], F32, tag="mom")
                    mom2 = mom_ps.tile([1, HTOK], F32, tag="mom")
                    xts = []
                    for o in range(NK):
                        ps = op_ps.tile([128, HTOK], F32, tag="op")
                        for k in range(NK):
                            nc.tensor.matmul(
                                ps[:, :],
                                wo_sb[:, k, o * 128:(o + 1) * 128],
                                ctx_own[:, k, :],
                                start=(k == 0),
                                stop=(k == NK - 1),
                            )
                        xt_o = xtl.tile([128, HTOK], mybir.dt.float32r, tag="xt_o")
                        nc.vector.scalar_tensor_tensor(
                            xt_o[:, :], ps[:, :], obg[:, o, 0:1], hT_sb[:, o, :],
                            op0=mybir.AluOpType.add, op1=mybir.AluOpType.add,
                        )
                        xts.append(xt_o)
                        sq = stp.tile([128, HTOK], BF16, tag="sq", bufs=4)
                        nc.gpsimd.tensor_mul(sq[:, :], xt_o[:, :], xt_o[:, :])
                        nc.tensor.matmul(
                            mom1[:, :], ones128r[:, :], xt_o[:, :],
                            start=(o == 0), stop=(o == NK - 1),
                            skip_group_check=True,
                        )
                        nc.tensor.matmul(
                            mom2[:, :], ones128[:, :], sq[:, :],
                            start=(o == 0), stop=(o == NK - 1),
                            skip_group_check=True,
                        )
                    muZ = lnp.tile([1, HTOK], F32, tag="muz", bufs=2)
                    m2Z = lnp.tile([1, HTOK], F32, tag="m2z", bufs=2)
                    nc.vector.tensor_scalar_mul(muZ[:, :], mom1[:, :], 1.0 / H)
                    nc.vector.tensor_scalar_mul(m2Z[:, :], mom2[:, :], 1.0 / H)
                    mu_b = lnp.tile([128, HTOK], F32, tag="mub", bufs=2)
                    m2_b = lnp.tile([128, HTOK], F32, tag="m2b", bufs=2)
                    nc.gpsimd.partition_broadcast(mu_b[:, :], muZ[:, :], channels=128)
                    nc.gpsimd.partition_broadcast(m2_b[:, :], m2Z[:, :], channels=128)
                    musq = lnp.tile([128, HTOK], F32, tag="musq", bufs=2)
                    nc.vector.tensor_mul(musq[:, :], mu_b[:, :], mu_b[:, :])
                    vare = lnp.tile([128, HTOK], F32, tag="vare", bufs=2)
                    nc.vector.tensor_sub(vare[:, :], m2_b[:, :], musq[:, :])
                    vare2 = lnp.tile([128, HTOK], F32, tag="vare2", bufs=2)
                    nc.vector.tensor_scalar_add(vare2[:, :], vare[:, :], 1e-12)
                    rvar = lnp.tile([128, HTOK], F32, tag="rvar", bufs=2)
                    nc.vector.reciprocal_approx_fast(rvar[:, :], vare2[:, :])
                    rstd = lnp.tile([128, HTOK], F32, tag="rstd", bufs=2)
                    nc.scalar.activation(
                        rstd[:, :], rvar[:, :], mybir.ActivationFunctionType.Sqrt
                    )
                    for o in range(NK):
                        eng = nc.vector if o % 2 == 0 else nc.gpsimd
                        dcen = stp.tile([128, HTOK], F32, tag="dcen", bufs=4)
                        eng.tensor_sub(dcen[:, :], xts[o][:, :], mu_b[:, :])
                        en = stp.tile([128, HTOK], F32, tag="en", bufs=4)
                        eng.tensor_mul(en[:, :], dcen[:, :], rstd[:, :])
                        outt = stp.tile([128, HTOK], F32, tag="outt", bufs=4)
                        eng.tensor_scalar(
                            outt[:, :], en[:, :], obg[:, o, 1:2], obg[:, o, 2:3],
                            op0=mybir.AluOpType.mult, op1=mybir.AluOpType.add,
                        )
                        nc.sync.dma_start(
                            out=y[o * 128:(o + 1) * 128, hs], in_=outt[:, :]
                        )

    nc.compile()
    return nc


def get_nc():
    if "nc" not in _NC_CACHE:
        _NC_CACHE["nc"] = build_nc()
    return _NC_CACHE["nc"]


def _owned_tokens(c):
    return list(range(HTOK * c, HTOK * (c + 1))) + list(
        range(S + HTOK * c, S + HTOK * (c + 1))
    )


def prepare_in_maps(inputs, fp8=True):
    hidden = np.asarray(inputs["hidden_states"], dtype=np.float32)
    mask = np.asarray(inputs["attention_mask"], dtype=np.float32)
    Wq = np.asarray(inputs["Wq"], dtype=np.float32)
    Wk = np.asarray(inputs["Wk"], dtype=np.float32)
    Wv = np.asarray(inputs["Wv"], dtype=np.float32)
    Wo = np.asarray(inputs["Wo"], dtype=np.float32)
    bq = np.asarray(inputs["bq"], dtype=np.float32)
    bk = np.asarray(inputs["bk"], dtype=np.float32)
    bv = np.asarray(inputs["bv"], dtype=np.float32)
    bo = np.asarray(inputs["bo"], dtype=np.float32)
    gamma = np.asarray(inputs["ln_gamma"], dtype=np.float32)
    beta = np.asarray(inputs["ln_beta"], dtype=np.float32)

    X = hidden.reshape(T, H)
    xT = np.ascontiguousarray(X.T)                      # [H, T] f32
    maskT_np = np.ascontiguousarray(mask.reshape(B, S))
    sact = 0.125 / (WS * WS) if fp8 else 0.125
    maskD_np = np.ascontiguousarray(
        maskT_np * (128.0 * LOG2E) + np.float32(SCHRAUD_C)
    ).astype(np.float32)
    wo_bf = Wo.astype(BF)

    def dr_pack(A, ncols):
        # [H, ncols] -> [64, NK, 2, ncols] with h = kk*128 + g*64 + r
        return np.ascontiguousarray(
            A.reshape(NK, 2, 64, ncols).transpose(2, 0, 1, 3)
        )

    if fp8:
        xdr = dr_pack(xT.astype(F8).astype(np.float32), T).astype(F8)
    else:
        xdr = xT.astype(BF)

    in_maps = []
    for c in range(W):
        csl = slice(CPC * c, CPC * (c + 1))
        toks = _owned_tokens(c)
        if fp8:
            wq_c = dr_pack((Wq[:, csl] * WS).astype(F8).astype(np.float32), CPC).astype(F8)
            wk_c = dr_pack((Wk[:, csl] * WS).astype(F8).astype(np.float32), CPC).astype(F8)
            wv_c = dr_pack((Wv[:, csl] * WS).astype(F8).astype(np.float32), CPC).astype(F8)
            bq_c = np.ascontiguousarray(bq[csl]) * WS
            bk_c = np.ascontiguousarray(bk[csl]) * WS
            bv_c = np.ascontiguousarray(bv[csl]) * WS
        else:
            wq_c = np.ascontiguousarray(Wq[:, csl]).astype(BF)
            wk_c = np.ascontiguousarray(Wk[:, csl]).astype(BF)
            wv_c = np.ascontiguousarray(Wv[:, csl]).astype(BF)
            bq_c, bk_c, bv_c = (
                np.ascontiguousarray(bq[csl]),
                np.ascontiguousarray(bk[csl]),
                np.ascontiguousarray(bv[csl]),
            )
        in_maps.append({
            "xdr": xdr,
            "wq": wq_c, "wk": wk_c, "wv": wv_c,
            "bq": bq_c, "bk": bk_c, "bv": bv_c,
            "wo": wo_bf, "bo": bo, "gamma": gamma, "beta": beta,
            "hT": np.ascontiguousarray(xT[:, toks]),
            "maskT": maskT_np, "maskD": maskD_np,
        })
    return in_maps


def kernel(**inputs):
    in_maps = prepare_in_maps(inputs)
    nc = get_nc()
    res = run_bass_kernel_spmd(nc, in_maps, core_ids=list(range(W)))
    out_flat = np.empty((T, H), dtype=np.float32)
    for c in range(W):
        out_flat[_owned_tokens(c), :] = res.results[c]["y"].T
    return out_flat.reshape(B, S, H)
